# revision 5
# baseline (speedup 1.0000x reference)
"""Single-head attention (B=4, S=4096, E=2048, d=128) on 8 trn2 cores.

Sharding: core c handles (batch b = c//2, seq half h = c%2). Each core
projects q/k/v for its own 2048-row half; the pair (2b, 2b+1) exchanges
K and V halves via four pairwise AllGathers, fired as early as possible
and ordered by when the peer data is first consumed (K-q0, V-q0, K-q1,
V-q1). The CC engine executes collectives serially at ~10-13us per
512KB, so the attention pass is split so the late arrivals are consumed
last:
  pass X: per query block, own keys (kp 0-7) + peer quarter-0 keys
          (kp 8-11) — needs K-q0/V-q0 peer data at ~50/55us.
  pass Y: per query block, peer quarter-1 keys (kp 12-15) — needs
          K-q1/V-q1 at ~90us, far behind the exchange completion.

Projection order (vs the old K,V,Q interleave per quarter): K+V for
both quarters first (accumulating over streamed x chunks), then Q for
both quarters — this frees the K/V exchange to start ~15us earlier.
V-half transposes ([d,k] -> [k,d], PE identity transposes) interleave
into the following matmul stream. PSUM plan: ps_big 3x[128,1024]
rotates K0,V0,K1 -> V1,Q0,Q1; ps_acc/ps_small hold transposes during
projection and ps_o/ps_sum during passes.

Per block: all score matmuls first (scoresT[k,q], 2 per kp into one
[128 x 1024] PSUM tile), exp per kp (scale folded; no max subtraction:
scores are O(sigma~1)) split ACT/DVE (DVE runs the Schraudolph bf16
bit-trick, ~3% max rel err — numerically validated to 1/2 of tiles;
used on ~1/4), denominator subtree adds on DVE (leaf=4) with exact
ones-column matmuls after the PV matmuls in the PE FIFO.
Host: out = (out_T / sums).T per core, reassembled into [4,4096,128].
"""

import numpy as np
import ml_dtypes

import concourse.tile as tile
from concourse import bacc, mybir
from concourse.bass_utils import run_bass_kernel_spmd
from concourse.masks import make_identity

N_CORES = 8
B, S, E, D = 4, 4096, 2048, 128
HALF = S // 2  # queries / own keys per core
QB = 512  # query block (PSUM bank width in fp32)
SQ = 1024  # projection quarter width
SCALE = 1.0 / float(np.sqrt(D))

BF16 = mybir.dt.bfloat16
F32 = mybir.dt.float32
AF = mybir.ActivationFunctionType

_CACHE = {}


def _build():
    nc = bacc.Bacc(
        trn_type="TRN2", target_bir_lowering=False, debug=False, num_devices=N_CORES
    )

    x_d = nc.dram_tensor("xt", [E, HALF], BF16, kind="ExternalInput").ap()
    w_d = nc.dram_tensor(
        "w", [128, (E // 128) * 3 * D], BF16, kind="ExternalInput"
    ).ap()
    bias_d = nc.dram_tensor("bias_cols", [D, 3], F32, kind="ExternalInput").ap()
    peer_d = nc.dram_tensor("peer", [1, 1], mybir.dt.uint32, kind="ExternalInput").ap()
    out_d = nc.dram_tensor("out_t", [D, HALF], F32, kind="ExternalOutput").ap()
    sums_d = nc.dram_tensor("sums", [1, HALF], F32, kind="ExternalOutput").ap()

    NE = E // 128  # 16 e-chunks
    NQB = HALF // QB  # 4 query blocks
    GROUPS = [[2 * i, 2 * i + 1] for i in range(N_CORES // 2)]

    SCH_A = float(SCALE * (1 << 7) / np.log(2.0))
    SCH_B = float(127 * (1 << 7) + 0.5 - 5.59)

    with tile.TileContext(nc) as tc:
        with (
            tc.tile_pool(name="xt", bufs=32) as xt_pool,
            tc.tile_pool(name="wsb", bufs=1) as w_pool,
            tc.tile_pool(name="persist", bufs=1) as persist,
            tc.tile_pool(name="vtt", bufs=2) as vtt_pool,
            tc.tile_pool(name="exp", bufs=20) as exp_pool,
            tc.tile_pool(name="comb", bufs=10) as comb_pool,
            tc.tile_pool(name="osb", bufs=2) as osb_pool,
            tc.tile_pool(name="dram", bufs=1, space="DRAM") as dram_pool,
            tc.tile_pool(name="ps_big", bufs=3, space="PSUM") as ps_big,
            tc.tile_pool(name="ps_acc", bufs=1, space="PSUM") as ps_acc,
            tc.tile_pool(name="ps_small", bufs=1, space="PSUM") as ps_small,
        ):
            # ---- constants / small inputs ----
            bias_sb = persist.tile([D, 3], F32, tag="bias")
            nc.scalar.dma_start(bias_sb[:], bias_d[:])
            ones_col = persist.tile([128, 1], BF16, tag="ones")
            nc.gpsimd.memset(ones_col[:], 1.0)
            ident = persist.tile([128, 128], BF16, tag="ident")
            make_identity(nc, ident[:])

            # ---- w + x loads in consumption order. x: per quarter, even e
            # on sync, odd on scalar (the two HWDGE rings). w piece 0 is
            # e0-only so the first matmuls start as early as possible.
            w_sb = w_pool.tile([128, NE * 3 * D], BF16, tag="w")
            we = 3 * D
            wg = NE * 3 * D // 4  # w quarter piece: 4 e-chunks
            xt = {}
            nc.sync.dma_start(w_sb[:, 0:we], w_d[:, 0:we])
            nc.scalar.dma_start(w_sb[:, wg : 2 * wg], w_d[:, wg : 2 * wg])

            def load_x(sq, e):
                eng = nc.sync if e % 2 == 0 else nc.scalar
                t = xt_pool.tile([128, SQ], BF16, tag="xt", name=f"xt{sq}_{e}")
                eng.dma_start(t[:], x_d[e * 128 : (e + 1) * 128, sq * SQ : (sq + 1) * SQ])
                xt[(sq, e)] = t

            for e in range(NE):
                if e == 2:
                    nc.sync.dma_start(w_sb[:, we:wg], w_d[:, we:wg])
                if e == 4 or e == 5:
                    g = e - 2
                    eng = nc.sync if e % 2 == 0 else nc.scalar
                    eng.dma_start(
                        w_sb[:, g * wg : (g + 1) * wg], w_d[:, g * wg : (g + 1) * wg]
                    )
                load_x(0, e)
            # first 6 quarter-1 loads issued here; the rest after the q0
            # evacuations so those aren't queued behind ring-credit-gated
            # DMA issues (same reasoning as the measured +2.5us in the old
            # layout).
            for e in range(6):
                load_x(1, e)

            # peer slot register (host supplies 1 on even cores, 0 on odd)
            peer_reg = nc.sync.alloc_register("peer_slot")
            nc.sync.reg_load(peer_reg, peer_d[0:1, 0:1])
            peer_val = nc.sync.snap(peer_reg, donate=True, min_val=0, max_val=1)

            qT = persist.tile([D, HALF], BF16, tag="qT")
            k_all = persist.tile([D, S], BF16, tag="k_all")  # [k own | k peer]
            v_sb = persist.tile([128, S // 128 * D], BF16, tag="v")  # own | peer
            sums_sb = persist.tile([1, HALF], F32, tag="sums_sb")
            o_stage = persist.tile([D, HALF], F32, tag="o_stage")

            # ---- collective staging (DRAM) ----
            cc_in = {}
            cc_out = {}
            for nm, shp in (
                ("k0", [D, SQ]),
                ("v0", [128, 8 * D]),
                ("k1", [D, SQ]),
                ("v1", [128, 8 * D]),
            ):
                cc_in[nm] = dram_pool.tile(shp, BF16, tag=f"cc_in_{nm}", name=f"cc_in_{nm}")
                cc_out[nm] = dram_pool.tile([2] + shp, BF16, tag=f"cc_out_{nm}", name=f"cc_out_{nm}")

            def exchange(nm, src_ap):
                nc.sync.dma_start(cc_in[nm][:], src_ap)
                nc.gpsimd.collective_compute(
                    "AllGather",
                    mybir.AluOpType.bypass,
                    replica_groups=GROUPS,
                    ins=[cc_in[nm].opt()],
                    outs=[cc_out[nm].opt()],
                )

            # ---- projection: K+V first (both quarters), then Q ----
            ps_k = [None, None]
            ps_v = [None, None]
            ps_k[0] = ps_big.tile([128, SQ], F32, tag="ps_big", name="ps_k0")
            ps_v[0] = ps_big.tile([128, SQ], F32, tag="ps_big", name="ps_v0")

            def kv_mms(sq, e):
                for g, ps in ((1, ps_k[sq]), (2, ps_v[sq])):
                    w_ap = w_sb[:, e * 3 * D + g * D : e * 3 * D + (g + 1) * D]
                    for half in range(2):
                        nc.tensor.matmul(
                            ps[:, half * QB : (half + 1) * QB],
                            lhsT=w_ap,
                            rhs=xt[(sq, e)][:, half * QB : (half + 1) * QB],
                            start=(e == 0),
                            stop=(e == NE - 1),
                        )

            vt_tmp = [None, None]

            def transpose_unit(sq, j):
                """One [128,128] PE transpose of vt_tmp[sq] -> v_sb chunk."""
                pool, ptag = (ps_acc, "ps_acc") if j % 2 == 0 else (ps_small, "ps_small")
                ps_t = pool.tile([128, 128], BF16, tag=ptag)
                nc.tensor.transpose(
                    ps_t[:], vt_tmp[sq][:, j * 128 : (j + 1) * 128], ident[:]
                )
                k = sq * 8 + j
                nc.vector.tensor_copy(v_sb[:, k * D : (k + 1) * D], ps_t[:])

            for e in range(NE):
                kv_mms(0, e)
            # q0 evacuations: K first (gates the first exchange), then V
            nc.scalar.activation(
                k_all[:, 0:SQ], ps_k[0][:], AF.Identity, bias=bias_sb[:, 1:2]
            )
            vt_tmp[0] = vtt_pool.tile([128, SQ], BF16, tag="vtt", name="vtt0")
            nc.scalar.activation(
                vt_tmp[0][:], ps_v[0][:], AF.Identity, bias=bias_sb[:, 2:3]
            )
            exchange("k0", k_all[:, 0:SQ])

            # quarter-1 K/V accumulation; V-q0 transposes interleaved.
            # V1 takes K0's freed ps_big slot, so alloc order matters.
            ps_k[1] = ps_big.tile([128, SQ], F32, tag="ps_big", name="ps_k1")
            ps_v[1] = ps_big.tile([128, SQ], F32, tag="ps_big", name="ps_v1")
            for e_l in range(6, NE):
                load_x(1, e_l)
            for e in range(NE):
                kv_mms(1, e)
                if e < 8:
                    transpose_unit(0, e)
            nc.scalar.activation(
                k_all[:, SQ:HALF], ps_k[1][:], AF.Identity, bias=bias_sb[:, 1:2]
            )
            vt_tmp[1] = vtt_pool.tile([128, SQ], BF16, tag="vtt", name="vtt1")
            nc.scalar.activation(
                vt_tmp[1][:], ps_v[1][:], AF.Identity, bias=bias_sb[:, 2:3]
            )
            exchange("v0", v_sb[:, 0 : 8 * D])
            exchange("k1", k_all[:, SQ:HALF])

            # ---- Q projection (both quarters); V-q1 transposes interleave
            ps_q0 = ps_big.tile([128, SQ], F32, tag="ps_big")
            ps_q1 = ps_big.tile([128, SQ], F32, tag="ps_big")
            for e in range(NE):
                w_ap = w_sb[:, e * 3 * D : e * 3 * D + D]
                for sq, ps in ((0, ps_q0), (1, ps_q1)):
                    for half in range(2):
                        nc.tensor.matmul(
                            ps[:, half * QB : (half + 1) * QB],
                            lhsT=w_ap,
                            rhs=xt[(sq, e)][:, half * QB : (half + 1) * QB],
                            start=(e == 0),
                            stop=(e == NE - 1),
                        )
                if e < 8:
                    transpose_unit(1, e)
            exchange("v1", v_sb[:, 8 * D : 16 * D])
            nc.scalar.activation(
                qT[:, 0:SQ], ps_q0[:], AF.Identity, bias=bias_sb[:, 0:1]
            )
            nc.scalar.activation(
                qT[:, SQ:HALF], ps_q1[:], AF.Identity, bias=bias_sb[:, 0:1]
            )

            # peer halves: DMA collective outputs into SBUF as they land
            nc.sync.dma_start(k_all[:, HALF : HALF + SQ], cc_out["k0"][peer_val])
            nc.sync.dma_start(v_sb[:, 16 * D : 24 * D], cc_out["v0"][peer_val])
            nc.sync.dma_start(k_all[:, HALF + SQ : S], cc_out["k1"][peer_val])
            nc.sync.dma_start(v_sb[:, 24 * D : 32 * D], cc_out["v1"][peer_val])

            # ---- attention passes ----
            def scores_exp(qb, kp, on_dve):
                q_ap = qT[:, qb * QB : (qb + 1) * QB]
                ps_s = ps_big.tile([128, 2 * QB], F32, tag="ps_big")
                for half in range(2):
                    k = 2 * kp + half
                    nc.tensor.matmul(
                        ps_s[:, half * QB : (half + 1) * QB],
                        lhsT=k_all[:, k * 128 : (k + 1) * 128],
                        rhs=q_ap,
                        start=True,
                        stop=True,
                    )
                ex = exp_pool.tile([128, 2 * QB], BF16, tag="exp")
                if on_dve:
                    nc.vector.tensor_scalar(
                        ex[:].bitcast(mybir.dt.int16),
                        ps_s[:],
                        SCH_A,
                        SCH_B,
                        mybir.AluOpType.mult,
                        mybir.AluOpType.add,
                    )
                else:
                    nc.scalar.activation(ex[:], ps_s[:], AF.Exp, scale=SCALE)
                return ex

            def subtree(exs):
                """DVE pair-add tree over whole [128, 2*QB] exp tiles; returns
                the root tile (un-folded; the ones-matmul takes both halves)."""
                level = list(exs)
                while len(level) > 1:
                    nxt = []
                    for i in range(0, len(level), 2):
                        if i + 1 < len(level):
                            comb = comb_pool.tile([128, 2 * QB], BF16, tag="comb")
                            nc.vector.tensor_add(comb[:], level[i][:], level[i + 1][:])
                            nxt.append(comb)
                        else:
                            nxt.append(level[i])
                    level = nxt
                return level[0]

            def block(qb, kp0, nkp, dve_set, leaf, first, last):
                """scores+exp+PV+denominator for query block qb, k-pairs
                [kp0, kp0+nkp)."""
                # subtree adds emitted right after each leaf group's last exp:
                # the DVE queue is strict FIFO, so batching them at the end
                # would delay the denominator roots past the PV matmuls and
                # stall the PE's ones-matmuls.
                exs = []
                roots = []
                for i, kp in enumerate(range(kp0, kp0 + nkp)):
                    exs.append(scores_exp(qb, kp, on_dve=(kp in dve_set)))  # tile named inside
                    if (i + 1) % leaf == 0:
                        roots.append(subtree(exs[i + 1 - leaf : i + 1]))
                ps_o = ps_acc.tile([128, QB], F32, tag="ps_acc")
                ps_sum = ps_small.tile([1, QB], F32, tag="ps_small")
                for i, kp in enumerate(range(kp0, kp0 + nkp)):
                    for half in range(2):
                        k = 2 * kp + half
                        nc.tensor.matmul(
                            ps_o[:],
                            lhsT=v_sb[:, k * D : (k + 1) * D],
                            rhs=exs[i][:, half * QB : (half + 1) * QB],
                            start=(i == 0 and half == 0),
                            stop=(i == nkp - 1 and half == 1),
                        )
                for ri, root in enumerate(roots):
                    for half in range(2):
                        nc.tensor.matmul(
                            ps_sum[:],
                            lhsT=ones_col[:],
                            rhs=root[:, half * QB : (half + 1) * QB],
                            start=(ri == 0 and half == 0),
                            stop=(ri == len(roots) - 1 and half == 1),
                        )
                o_sl = o_stage[:, qb * QB : (qb + 1) * QB]
                s_sl = sums_sb[:, qb * QB : (qb + 1) * QB]
                if first:
                    nc.scalar.activation(o_sl, ps_o[:], AF.Identity)
                    nc.vector.tensor_copy(s_sl, ps_sum[:])
                else:
                    o_out = osb_pool.tile([128, QB], F32, tag="osb")
                    nc.vector.tensor_add(o_out[:], o_sl, ps_o[:])
                    nc.vector.tensor_add(s_sl, s_sl, ps_sum[:])
                if last:
                    nc.sync.dma_start(out_d[:, qb * QB : (qb + 1) * QB], o_out[:])
                    nc.sync.dma_start(sums_d[:, qb * QB : (qb + 1) * QB], s_sl)

            # pass X: own keys + peer quarter 0 (12 k-pairs per block)
            for qb in range(NQB):
                block(
                    qb, 0, 12, dve_set={2, 6, 10}, leaf=4, first=True, last=False
                )
            # pass Y: peer quarter 1 (4 k-pairs per block)
            for qb in range(NQB):
                dve = {14} if qb < NQB - 1 else {13, 15}
                block(qb, 12, 4, dve_set=dve, leaf=4, first=False, last=True)

    nc.compile()
    return nc


def _prep_inputs(x, W, b):
    """Host-side sharding prep: cast bf16, transpose to xT, slice halves."""
    b_f = np.asarray(b, dtype=np.float32)
    bias_cols = np.ascontiguousarray(b_f.reshape(3, D).T)  # [128, 3]
    w_bf = np.ascontiguousarray(
        np.asarray(W)
        .astype(ml_dtypes.bfloat16)
        .reshape(E // 128, 128, 3 * D)
        .transpose(1, 0, 2)
        .reshape(128, (E // 128) * 3 * D)
    )
    in_maps = []
    for bb in range(B):
        xt_full = np.ascontiguousarray(
            np.asarray(x[bb]).astype(ml_dtypes.bfloat16).T
        )  # [E, S]
        for h in range(2):
            xc = np.ascontiguousarray(xt_full[:, h * HALF : (h + 1) * HALF])
            peer = np.array([[1 - h]], dtype=np.uint32)
            in_maps.append(
                {"xt": xc, "w": w_bf, "bias_cols": bias_cols, "peer": peer}
            )
    return in_maps


def _run(in_maps, trace=False, trace_kwargs=None):
    if "nc" not in _CACHE:
        _CACHE["nc"] = _build()
    return run_bass_kernel_spmd(
        _CACHE["nc"],
        in_maps,
        list(range(N_CORES)),
        trace=trace,
        **(trace_kwargs or {}),
    )


def kernel(x, W, b):
    in_maps = _prep_inputs(x, W, b)
    res = None
    for attempt in range(3):
        try:
            res = _run(in_maps)
            break
        except Exception:
            if attempt == 2:
                raise
    out = np.empty((B, S, D), dtype=np.float32)
    for c in range(N_CORES):
        bb, h = c // 2, c % 2
        o_t = res.results[c]["out_t"]  # [D, HALF]
        sums = res.results[c]["sums"]  # [1, HALF]
        out[bb, h * HALF : (h + 1) * HALF, :] = (o_t / sums).T
    return out


# revision 7
# speedup vs baseline: 1.0176x; 1.0176x over previous
"""Single-head attention (B=4, S=4096, E=2048, d=128) on 8 trn2 cores.

Sharding: core c handles (batch b = c//2, seq half h = c%2). Each core
projects q/k/v for its own 2048-row half; the pair (2b, 2b+1) exchanges
K and V halves via four pairwise AllGathers fired in consumption order
(K-q0, V-q0, K-q1, V-q1). A tiny dummy AllGather at kernel start pays
the ~11.5us first-collective CC latency while the x DMAs stream.

DMA queue plan (the two HWDGE rings shared ~360GB/s in measurement):
  scalar ring: bias, one w piece, x q0-odd chunks — all done by ~21us
    so the ACT queue is free for the K0/V0 evacuations that gate the
    exchange (leaving later DMA issues here measurably delays them).
  sync ring: w rest, x q0-even + q1-even, collective staging in/out,
    output stores.
  gpsimd SWDGE: x q1-odd chunks (third parallel path), collective
    triggers.

Projection: K+V accumulate first (both quarters, per streamed e-chunk),
then Q — so the K/V exchange starts ~15us earlier than a K,V,Q
interleave. V-half transposes ([d,k]->[k,d] PE identity transposes)
interleave into the following matmul stream. PSUM: ps_big 3x[128,1024]
rotates K0,V0,K1 -> V1,Q0,Q1 -> score tiles; ps_acc/ps_small hold
transposes during projection, ps_o/ps_sum during the passes.

Attention: pass X = own keys (kp 0-7), pass Y = peer keys (kp 8-15,
o_stage accumulation) — Y starts ~30us after the last exchange lands,
so collective jitter never stalls the PE. Blocks are software-pipelined:
block n's score matmuls interleave with block n-1's PV matmuls in the
PE FIFO, so the exp engines (ACT 6 + DVE-Schraudolph 2 per block,
validated to 1/4 total) stay under the PE stage time, and PSUM score
tiles recycle with slack. Denominators: DVE pair-add subtrees (leaf=4)
emitted at leaf boundaries + exact ones-column matmuls after the PVs;
output/sum evacuations on the DVE (the ACT queue is exp-bound).
Host: out = (out_T / sums).T per core, reassembled into [4,4096,128].
"""

import numpy as np
import ml_dtypes

import concourse.tile as tile
from concourse import bacc, mybir
from concourse.bass_utils import run_bass_kernel_spmd
from concourse.masks import make_identity

N_CORES = 8
B, S, E, D = 4, 4096, 2048, 128
HALF = S // 2  # queries / own keys per core
QB = 512  # query block (PSUM bank width in fp32)
SQ = 1024  # projection quarter width
SCALE = 1.0 / float(np.sqrt(D))

BF16 = mybir.dt.bfloat16
F32 = mybir.dt.float32
AF = mybir.ActivationFunctionType

_CACHE = {}


def _build():
    nc = bacc.Bacc(
        trn_type="TRN2", target_bir_lowering=False, debug=False, num_devices=N_CORES
    )

    x_d = nc.dram_tensor("xt", [E, HALF], BF16, kind="ExternalInput").ap()
    w_d = nc.dram_tensor(
        "w", [128, (E // 128) * 3 * D], BF16, kind="ExternalInput"
    ).ap()
    bias_d = nc.dram_tensor("bias_cols", [D, 3], F32, kind="ExternalInput").ap()
    peer_d = nc.dram_tensor("peer", [1, 1], mybir.dt.uint32, kind="ExternalInput").ap()
    out_d = nc.dram_tensor("out_t", [D, HALF], F32, kind="ExternalOutput").ap()
    sums_d = nc.dram_tensor("sums", [1, HALF], F32, kind="ExternalOutput").ap()

    NE = E // 128  # 16 e-chunks
    NQB = HALF // QB  # 4 query blocks
    GROUPS = [[2 * i, 2 * i + 1] for i in range(N_CORES // 2)]

    SCH_A = float(SCALE * (1 << 7) / np.log(2.0))
    SCH_B = float(127 * (1 << 7) + 0.5 - 5.59)

    with tile.TileContext(nc) as tc:
        with (
            tc.tile_pool(name="xt", bufs=32) as xt_pool,
            tc.tile_pool(name="wsb", bufs=1) as w_pool,
            tc.tile_pool(name="persist", bufs=1) as persist,
            tc.tile_pool(name="vtt", bufs=2) as vtt_pool,
            tc.tile_pool(name="exp", bufs=20) as exp_pool,
            tc.tile_pool(name="comb", bufs=8) as comb_pool,
            tc.tile_pool(name="osb", bufs=2) as osb_pool,
            tc.tile_pool(name="dram", bufs=1, space="DRAM") as dram_pool,
            tc.tile_pool(name="ps_big", bufs=3, space="PSUM") as ps_big,
            tc.tile_pool(name="ps_acc", bufs=1, space="PSUM") as ps_acc,
            tc.tile_pool(name="ps_small", bufs=1, space="PSUM") as ps_small,
        ):
            # ---- constants ----
            bias_sb = persist.tile([D, 3], F32, tag="bias")
            nc.scalar.dma_start(bias_sb[:], bias_d[:])
            ones_col = persist.tile([128, 1], BF16, tag="ones")
            nc.gpsimd.memset(ones_col[:], 1.0)
            ident = persist.tile([128, 128], BF16, tag="ident")
            make_identity(nc, ident[:])

            # ---- CC warmup: the first collective after CC idle pays ~11.5us
            # of mesh-algo startup before any data moves. Pay it now, on
            # garbage bytes, while the x stream owns the DMA rings. The
            # input DRAM tile is deliberately never written: AllGather
            # bypass copies whatever is there; only timing matters.
            warm_in = dram_pool.tile([1, 2], BF16, tag="warm_in")
            warm_out = dram_pool.tile([2, 1, 2], BF16, tag="warm_out")
            nc.gpsimd.collective_compute(
                "AllGather",
                mybir.AluOpType.bypass,
                replica_groups=GROUPS,
                ins=[warm_in.opt()],
                outs=[warm_out.opt()],
            )

            # ---- w + x loads in consumption order ----
            w_sb = w_pool.tile([128, NE * 3 * D], BF16, tag="w")
            we = 3 * D
            wg = NE * 3 * D // 4  # w quarter piece: 4 e-chunks
            xt = {}
            nc.sync.dma_start(w_sb[:, 0:we], w_d[:, 0:we])
            nc.scalar.dma_start(w_sb[:, wg : 2 * wg], w_d[:, wg : 2 * wg])

            def load_x(sq, e):
                if sq == 1 and e % 2 == 1:
                    eng = nc.gpsimd  # SWDGE: third parallel DMA path
                elif e % 2 == 0:
                    eng = nc.sync
                else:
                    eng = nc.scalar
                t = xt_pool.tile([128, SQ], BF16, tag="xt", name=f"xt{sq}_{e}")
                eng.dma_start(
                    t[:], x_d[e * 128 : (e + 1) * 128, sq * SQ : (sq + 1) * SQ]
                )
                xt[(sq, e)] = t

            for e in range(NE):
                if e == 2:
                    nc.sync.dma_start(w_sb[:, we:wg], w_d[:, we:wg])
                if e == 4 or e == 5:
                    g = e - 2
                    nc.sync.dma_start(
                        w_sb[:, g * wg : (g + 1) * wg], w_d[:, g * wg : (g + 1) * wg]
                    )
                load_x(0, e)
            for e in range(NE):
                load_x(1, e)

            # peer slot register (host supplies 1 on even cores, 0 on odd)
            peer_reg = nc.sync.alloc_register("peer_slot")
            nc.sync.reg_load(peer_reg, peer_d[0:1, 0:1])
            peer_val = nc.sync.snap(peer_reg, donate=True, min_val=0, max_val=1)

            qT = persist.tile([D, HALF], BF16, tag="qT")
            k_all = persist.tile([D, S], BF16, tag="k_all")  # [k own | k peer]
            v_sb = persist.tile([128, S // 128 * D], BF16, tag="v")  # own | peer
            sums_sb = persist.tile([1, HALF], F32, tag="sums_sb")
            o_stage = persist.tile([D, HALF], F32, tag="o_stage")

            # ---- collective staging (DRAM) ----
            cc_in = {}
            cc_out = {}
            for nm, shp in (
                ("k0", [D, SQ]),
                ("v0", [128, 8 * D]),
                ("k1", [D, SQ]),
                ("v1", [128, 8 * D]),
            ):
                cc_in[nm] = dram_pool.tile(
                    shp, BF16, tag=f"cc_in_{nm}", name=f"cc_in_{nm}"
                )
                cc_out[nm] = dram_pool.tile(
                    [2] + shp, BF16, tag=f"cc_out_{nm}", name=f"cc_out_{nm}"
                )

            def exchange(nm, src_ap):
                nc.sync.dma_start(cc_in[nm][:], src_ap)
                nc.gpsimd.collective_compute(
                    "AllGather",
                    mybir.AluOpType.bypass,
                    replica_groups=GROUPS,
                    ins=[cc_in[nm].opt()],
                    outs=[cc_out[nm].opt()],
                )

            # ---- projection: K+V first (both quarters), then Q ----
            ps_k = [None, None]
            ps_v = [None, None]
            ps_k[0] = ps_big.tile([128, SQ], F32, tag="ps_big", name="ps_k0")
            ps_v[0] = ps_big.tile([128, SQ], F32, tag="ps_big", name="ps_v0")

            def kv_mms(sq, e):
                for g, ps in ((1, ps_k[sq]), (2, ps_v[sq])):
                    w_ap = w_sb[:, e * 3 * D + g * D : e * 3 * D + (g + 1) * D]
                    for half in range(2):
                        nc.tensor.matmul(
                            ps[:, half * QB : (half + 1) * QB],
                            lhsT=w_ap,
                            rhs=xt[(sq, e)][:, half * QB : (half + 1) * QB],
                            start=(e == 0),
                            stop=(e == NE - 1),
                        )

            vt_tmp = [None, None]

            def transpose_unit(sq, j):
                """One [128,128] PE transpose of vt_tmp[sq] -> v_sb chunk."""
                pool, ptag = (ps_acc, "ps_acc") if j % 2 == 0 else (ps_small, "ps_small")
                ps_t = pool.tile([128, 128], BF16, tag=ptag)
                nc.tensor.transpose(
                    ps_t[:], vt_tmp[sq][:, j * 128 : (j + 1) * 128], ident[:]
                )
                k = sq * 8 + j
                nc.vector.tensor_copy(v_sb[:, k * D : (k + 1) * D], ps_t[:])

            for e in range(NE):
                kv_mms(0, e)
            # q0 evacuations: K first (gates the first exchange), then V
            nc.scalar.activation(
                k_all[:, 0:SQ], ps_k[0][:], AF.Identity, bias=bias_sb[:, 1:2]
            )
            vt_tmp[0] = vtt_pool.tile([128, SQ], BF16, tag="vtt", name="vtt0")
            nc.scalar.activation(
                vt_tmp[0][:], ps_v[0][:], AF.Identity, bias=bias_sb[:, 2:3]
            )
            exchange("k0", k_all[:, 0:SQ])

            # quarter-1 K/V accumulation; V-q0 transposes interleaved.
            # V1 takes K0's freed ps_big slot, so alloc order matters.
            ps_k[1] = ps_big.tile([128, SQ], F32, tag="ps_big", name="ps_k1")
            ps_v[1] = ps_big.tile([128, SQ], F32, tag="ps_big", name="ps_v1")
            for e in range(NE):
                kv_mms(1, e)
                if e < 8:
                    transpose_unit(0, e)
            nc.scalar.activation(
                k_all[:, SQ:HALF], ps_k[1][:], AF.Identity, bias=bias_sb[:, 1:2]
            )
            vt_tmp[1] = vtt_pool.tile([128, SQ], BF16, tag="vtt", name="vtt1")
            nc.scalar.activation(
                vt_tmp[1][:], ps_v[1][:], AF.Identity, bias=bias_sb[:, 2:3]
            )
            exchange("v0", v_sb[:, 0 : 8 * D])
            exchange("k1", k_all[:, SQ:HALF])

            # ---- Q projection (both quarters); V-q1 transposes interleave
            ps_q0 = ps_big.tile([128, SQ], F32, tag="ps_big", name="ps_q0")
            ps_q1 = ps_big.tile([128, SQ], F32, tag="ps_big", name="ps_q1")
            for e in range(NE):
                w_ap = w_sb[:, e * 3 * D : e * 3 * D + D]
                for sq, ps in ((0, ps_q0), (1, ps_q1)):
                    for half in range(2):
                        nc.tensor.matmul(
                            ps[:, half * QB : (half + 1) * QB],
                            lhsT=w_ap,
                            rhs=xt[(sq, e)][:, half * QB : (half + 1) * QB],
                            start=(e == 0),
                            stop=(e == NE - 1),
                        )
                if e < 8:
                    transpose_unit(1, e)
            exchange("v1", v_sb[:, 8 * D : 16 * D])
            nc.scalar.activation(
                qT[:, 0:SQ], ps_q0[:], AF.Identity, bias=bias_sb[:, 0:1]
            )
            nc.scalar.activation(
                qT[:, SQ:HALF], ps_q1[:], AF.Identity, bias=bias_sb[:, 0:1]
            )

            # peer halves: DMA collective outputs into SBUF as they land
            nc.sync.dma_start(k_all[:, HALF : HALF + SQ], cc_out["k0"][peer_val])
            nc.sync.dma_start(v_sb[:, 16 * D : 24 * D], cc_out["v0"][peer_val])
            nc.sync.dma_start(k_all[:, HALF + SQ : S], cc_out["k1"][peer_val])
            nc.sync.dma_start(v_sb[:, 24 * D : 32 * D], cc_out["v1"][peer_val])

            # ---- attention: software-pipelined blocks ----
            def scores_exp(qb, kp, on_dve):
                q_ap = qT[:, qb * QB : (qb + 1) * QB]
                ps_s = ps_big.tile([128, 2 * QB], F32, tag="ps_big")
                for half in range(2):
                    k = 2 * kp + half
                    nc.tensor.matmul(
                        ps_s[:, half * QB : (half + 1) * QB],
                        lhsT=k_all[:, k * 128 : (k + 1) * 128],
                        rhs=q_ap,
                        start=True,
                        stop=True,
                    )
                ex = exp_pool.tile([128, 2 * QB], BF16, tag="exp")
                if on_dve:
                    nc.vector.tensor_scalar(
                        ex[:].bitcast(mybir.dt.int16),
                        ps_s[:],
                        SCH_A,
                        SCH_B,
                        mybir.AluOpType.mult,
                        mybir.AluOpType.add,
                    )
                else:
                    nc.scalar.activation(ex[:], ps_s[:], AF.Exp, scale=SCALE)
                return ex

            def subtree(exs):
                level = list(exs)
                while len(level) > 1:
                    nxt = []
                    for i in range(0, len(level), 2):
                        if i + 1 < len(level):
                            comb = comb_pool.tile([128, 2 * QB], BF16, tag="comb")
                            nc.vector.tensor_add(comb[:], level[i][:], level[i + 1][:])
                            nxt.append(comb)
                        else:
                            nxt.append(level[i])
                    level = nxt
                return level[0]

            NKP = 8  # k-pairs per block
            LEAF = 4
            blocks = [(qb, 0) for qb in range(NQB)] + [(qb, 8) for qb in range(NQB)]

            def emit_stage(cur, prev):
                """Interleave cur block's scores+exp with prev block's PV."""
                if prev is not None:
                    prev["ps_o"] = ps_acc.tile([128, QB], F32, tag="ps_acc", name="ps_o")
                    prev["ps_sum"] = ps_small.tile([1, QB], F32, tag="ps_small", name="ps_sum")
                for i in range(NKP):
                    if cur is not None:
                        qb, kp0 = cur["qb"], cur["kp0"]
                        kp = kp0 + i
                        # DVE exps at i in {2,6}: ~1/4 Schraudolph overall
                        cur["exs"].append(scores_exp(qb, kp, on_dve=(i in (2, 6))))
                        if (i + 1) % LEAF == 0:
                            cur["roots"].append(
                                subtree(cur["exs"][i + 1 - LEAF : i + 1])
                            )
                    if prev is not None:
                        kp = prev["kp0"] + i
                        for half in range(2):
                            k = 2 * kp + half
                            nc.tensor.matmul(
                                prev["ps_o"][:],
                                lhsT=v_sb[:, k * D : (k + 1) * D],
                                rhs=prev["exs"][i][:, half * QB : (half + 1) * QB],
                                start=(i == 0 and half == 0),
                                stop=(i == NKP - 1 and half == 1),
                            )
                if prev is None:
                    return
                for ri, root in enumerate(prev["roots"]):
                    for half in range(2):
                        nc.tensor.matmul(
                            prev["ps_sum"][:],
                            lhsT=ones_col[:],
                            rhs=root[:, half * QB : (half + 1) * QB],
                            start=(ri == 0 and half == 0),
                            stop=(ri == len(prev["roots"]) - 1 and half == 1),
                        )
                qb = prev["qb"]
                o_sl = o_stage[:, qb * QB : (qb + 1) * QB]
                s_sl = sums_sb[:, qb * QB : (qb + 1) * QB]
                if prev["kp0"] == 0:  # pass X: stage into SBUF
                    nc.vector.tensor_copy(o_sl, prev["ps_o"][:])
                    nc.vector.tensor_copy(s_sl, prev["ps_sum"][:])
                else:  # pass Y: combine + store
                    o_out = osb_pool.tile([128, QB], F32, tag="osb")
                    nc.vector.tensor_add(o_out[:], o_sl, prev["ps_o"][:])
                    nc.vector.tensor_add(s_sl, s_sl, prev["ps_sum"][:])
                    nc.sync.dma_start(out_d[:, qb * QB : (qb + 1) * QB], o_out[:])
                    nc.sync.dma_start(sums_d[:, qb * QB : (qb + 1) * QB], s_sl)

            prev = None
            for qb, kp0 in blocks:
                cur = {"qb": qb, "kp0": kp0, "exs": [], "roots": []}
                emit_stage(cur, prev)
                prev = cur
            emit_stage(None, prev)

    nc.compile()
    return nc


def _prep_inputs(x, W, b):
    """Host-side sharding prep: cast bf16, transpose to xT, slice halves."""
    b_f = np.asarray(b, dtype=np.float32)
    bias_cols = np.ascontiguousarray(b_f.reshape(3, D).T)  # [128, 3]
    w_bf = np.ascontiguousarray(
        np.asarray(W)
        .astype(ml_dtypes.bfloat16)
        .reshape(E // 128, 128, 3 * D)
        .transpose(1, 0, 2)
        .reshape(128, (E // 128) * 3 * D)
    )
    in_maps = []
    for bb in range(B):
        xt_full = np.ascontiguousarray(
            np.asarray(x[bb]).astype(ml_dtypes.bfloat16).T
        )  # [E, S]
        for h in range(2):
            xc = np.ascontiguousarray(xt_full[:, h * HALF : (h + 1) * HALF])
            peer = np.array([[1 - h]], dtype=np.uint32)
            in_maps.append(
                {"xt": xc, "w": w_bf, "bias_cols": bias_cols, "peer": peer}
            )
    return in_maps


def _run(in_maps, trace=False, trace_kwargs=None):
    if "nc" not in _CACHE:
        _CACHE["nc"] = _build()
    return run_bass_kernel_spmd(
        _CACHE["nc"],
        in_maps,
        list(range(N_CORES)),
        trace=trace,
        **(trace_kwargs or {}),
    )


def kernel(x, W, b):
    in_maps = _prep_inputs(x, W, b)
    res = None
    for attempt in range(3):
        try:
            res = _run(in_maps)
            break
        except Exception:
            if attempt == 2:
                raise
    out = np.empty((B, S, D), dtype=np.float32)
    for c in range(N_CORES):
        bb, h = c // 2, c % 2
        o_t = res.results[c]["out_t"]  # [D, HALF]
        sums = res.results[c]["sums"]  # [1, HALF]
        out[bb, h * HALF : (h + 1) * HALF, :] = (o_t / sums).T
    return out


# revision 9
# speedup vs baseline: 1.1283x; 1.1087x over previous
"""Single-head attention (B=4, S=4096, E=2048, d=128) on 8 trn2 cores.

Sharding: core c handles (batch b = c//2, seq half h = c%2). Each core
projects q/k/v for its own 2048-row half; the pair (2b, 2b+1) exchanges
K and V halves via four pairwise AllGathers. Measured CC behavior: the
first mesh cannot begin before ~52us regardless of trigger time (NRT
arming), then meshes run serially at ~7-8us per 256KB — so a dummy
warmup AllGather is fired at ~8us to absorb the arming latency, and the
attention pass is split own/peer (8/8 k-pairs) so peer data is first
consumed ~95us, far behind the worst-case exchange completion (~85us).

Engine/queue plan (all measured):
  sync HWDGE ring: w pieces 0/1, x even e-chunks (both quarters),
    output stores. scalar HWDGE ring: bias, w pieces 2-4, x odd
    e-chunks. Two rings share ~360GB/s; a third (gpsimd SWDGE) path
    starves the scalar ring, so x stays on two rings.
  ACT queue: exp only (plus a tiny warmup activation to preload the
    Exp table before the pass). Projection evacuations run on the DVE
    (tensor_scalar_add with the [128,1] bias column) — ACT evacuations
    behind ring-credit-gated DMA issues measurably slip by >10us.
  gpsimd queue: collective staging DMAs in AND out + triggers. (cc_out
    landings on the sync ring got statically scheduled after pass
    stores, stalling pass-Y PV by ~4us.)

Projection: per streamed x chunk, K, V, Q matmuls (6 x N=512) per
quarter — PE-bound at ~1.28us/chunk vs ~0.72us arrival. V-half
transposes ([d,k]->[k,d] PE identity transposes): quarter-0's ride the
quarter-1 matmul stream, quarter-1's ride the first pass stage.
PSUM: ps_big 3x[128,1024] holds K,V,Q of one quarter, rotating into
the next quarter then score tiles; ps_acc/ps_small hold transposes
during projection, ps_o/ps_sum during the passes.

Attention: blocks of 8 k-pairs x 512 queries, software-pipelined:
block n's score matmuls interleave with block n-1's PV matmuls in the
PE FIFO, so the exp engines (ACT 6 + DVE-Schraudolph 2 per block, ~3%
max rel err, numerically validated to 1/2 of tiles) stay under the
~7.9us PE stage time and score PSUM tiles recycle with slack.
Denominators: DVE pair-add subtrees (leaf=4) at leaf boundaries +
exact ones-column matmuls after the PVs. Output/sums evacuate on the
DVE; host divides and transposes.
"""

import numpy as np
import ml_dtypes

import concourse.tile as tile
from concourse import bacc, mybir
from concourse.bass_utils import run_bass_kernel_spmd
from concourse.masks import make_identity

N_CORES = 8
B, S, E, D = 4, 4096, 2048, 128
HALF = S // 2  # queries / own keys per core
QB = 512  # query block (PSUM bank width in fp32)
SQ = 1024  # projection quarter width
SCALE = 1.0 / float(np.sqrt(D))

BF16 = mybir.dt.bfloat16
F32 = mybir.dt.float32
AF = mybir.ActivationFunctionType

_CACHE = {}


def _build():
    nc = bacc.Bacc(
        trn_type="TRN2", target_bir_lowering=False, debug=False, num_devices=N_CORES
    )

    x_d = nc.dram_tensor("xt", [E, HALF], BF16, kind="ExternalInput").ap()
    w_d = nc.dram_tensor(
        "w", [128, (E // 128) * 3 * D], BF16, kind="ExternalInput"
    ).ap()
    bias_d = nc.dram_tensor("bias_cols", [D, 3], F32, kind="ExternalInput").ap()
    peer_d = nc.dram_tensor("peer", [1, 1], mybir.dt.uint32, kind="ExternalInput").ap()
    out_d = nc.dram_tensor("out_t", [D, HALF], F32, kind="ExternalOutput").ap()
    sums_d = nc.dram_tensor("sums", [1, HALF], F32, kind="ExternalOutput").ap()

    NE = E // 128  # 16 e-chunks
    NQB = HALF // QB  # 4 query blocks
    GROUPS = [[2 * i, 2 * i + 1] for i in range(N_CORES // 2)]

    SCH_A = float(SCALE * (1 << 7) / np.log(2.0))
    SCH_B = float(127 * (1 << 7) + 0.5 - 5.59)

    with tile.TileContext(nc) as tc:
        with (
            tc.tile_pool(name="xt", bufs=32) as xt_pool,
            tc.tile_pool(name="wsb", bufs=1) as w_pool,
            tc.tile_pool(name="persist", bufs=1) as persist,
            tc.tile_pool(name="vtt", bufs=2) as vtt_pool,
            tc.tile_pool(name="exp", bufs=20) as exp_pool,
            tc.tile_pool(name="comb", bufs=8) as comb_pool,
            tc.tile_pool(name="osb", bufs=2) as osb_pool,
            tc.tile_pool(name="dram", bufs=1, space="DRAM") as dram_pool,
            tc.tile_pool(name="ps_big", bufs=3, space="PSUM") as ps_big,
            tc.tile_pool(name="ps_acc", bufs=1, space="PSUM") as ps_acc,
            tc.tile_pool(name="ps_small", bufs=1, space="PSUM") as ps_small,
        ):
            # ---- constants ----
            bias_sb = persist.tile([D, 3], F32, tag="bias")
            nc.scalar.dma_start(bias_sb[:], bias_d[:])
            ones_col = persist.tile([128, 1], BF16, tag="ones")
            nc.gpsimd.memset(ones_col[:], 1.0)
            ident = persist.tile([128, 128], BF16, tag="ident")
            make_identity(nc, ident[:])
            # preload the ACT Exp table now (~1.3us) instead of at the
            # first pass exp
            act_warm = persist.tile([1, 1], BF16, tag="act_warm")
            nc.scalar.activation(act_warm[:], bias_sb[0:1, 0:1], AF.Exp, scale=1.0)

            # ---- CC warmup (see module docstring) ----
            warm_in = dram_pool.tile([1, 2], BF16, tag="warm_in")
            warm_out = dram_pool.tile([2, 1, 2], BF16, tag="warm_out")
            nc.gpsimd.collective_compute(
                "AllGather",
                mybir.AluOpType.bypass,
                replica_groups=GROUPS,
                ins=[warm_in.opt()],
                outs=[warm_out.opt()],
            )

            # ---- w + x loads in consumption order ----
            w_sb = w_pool.tile([128, NE * 3 * D], BF16, tag="w")
            we = 3 * D
            wg = NE * 3 * D // 4  # w quarter piece: 4 e-chunks
            xt = {}
            nc.sync.dma_start(w_sb[:, 0:we], w_d[:, 0:we])
            nc.scalar.dma_start(w_sb[:, wg : 2 * wg], w_d[:, wg : 2 * wg])

            def load_x(sq, e):
                eng = nc.sync if e % 2 == 0 else nc.scalar
                t = xt_pool.tile([128, SQ], BF16, tag="xt", name=f"xt{sq}_{e}")
                eng.dma_start(
                    t[:], x_d[e * 128 : (e + 1) * 128, sq * SQ : (sq + 1) * SQ]
                )
                xt[(sq, e)] = t

            for e in range(NE):
                if e == 2:
                    nc.sync.dma_start(w_sb[:, we:wg], w_d[:, we:wg])
                if e == 4 or e == 5:
                    g = e - 2
                    nc.scalar.dma_start(
                        w_sb[:, g * wg : (g + 1) * wg], w_d[:, g * wg : (g + 1) * wg]
                    )
                load_x(0, e)
            for e in range(NE):
                load_x(1, e)

            # peer slot register (host supplies 1 on even cores, 0 on odd).
            # Allocated on gpsimd: the peer landings are gpsimd SWDGE DMAs
            # and register APs are engine-scoped.
            peer_reg = nc.gpsimd.alloc_register("peer_slot")
            nc.gpsimd.reg_load(peer_reg, peer_d[0:1, 0:1])
            peer_val = nc.gpsimd.snap(peer_reg, donate=True, min_val=0, max_val=1)

            qT = persist.tile([D, HALF], BF16, tag="qT")
            k_all = persist.tile([D, S], BF16, tag="k_all")  # [k own | k peer]
            v_sb = persist.tile([128, S // 128 * D], BF16, tag="v")  # own | peer
            sums_sb = persist.tile([1, HALF], F32, tag="sums_sb")
            o_stage = persist.tile([D, HALF], F32, tag="o_stage")

            # ---- collective staging (DRAM) ----
            cc_in = {}
            cc_out = {}
            for nm, shp in (
                ("k0", [D, SQ]),
                ("v0", [128, 8 * D]),
                ("k1", [D, SQ]),
                ("v1", [128, 8 * D]),
            ):
                cc_in[nm] = dram_pool.tile(
                    shp, BF16, tag=f"cc_in_{nm}", name=f"cc_in_{nm}"
                )
                cc_out[nm] = dram_pool.tile(
                    [2] + shp, BF16, tag=f"cc_out_{nm}", name=f"cc_out_{nm}"
                )

            def exchange(nm, src_ap):
                nc.gpsimd.dma_start(cc_in[nm][:], src_ap)
                nc.gpsimd.collective_compute(
                    "AllGather",
                    mybir.AluOpType.bypass,
                    replica_groups=GROUPS,
                    ins=[cc_in[nm].opt()],
                    outs=[cc_out[nm].opt()],
                )

            # ---- projection: per chunk K, V, Q; quarter at a time ----
            vt_tmp = [None, None]

            def transpose_unit(sq, j):
                """One [128,128] PE transpose of vt_tmp[sq] -> v_sb chunk."""
                pool, ptag = (ps_acc, "ps_acc") if j % 2 == 0 else (ps_small, "ps_small")
                ps_t = pool.tile([128, 128], BF16, tag=ptag)
                nc.tensor.transpose(
                    ps_t[:], vt_tmp[sq][:, j * 128 : (j + 1) * 128], ident[:]
                )
                k = sq * 8 + j
                nc.vector.tensor_copy(v_sb[:, k * D : (k + 1) * D], ps_t[:])

            def dve_evac(dst_ap, ps, g):
                """PSUM -> SBUF with bias add, on the (projection-idle) DVE."""
                nc.vector.tensor_scalar_add(dst_ap, ps[:], bias_sb[:, g : g + 1])

            for sq in range(2):
                ps_k = ps_big.tile([128, SQ], F32, tag="ps_big", name=f"ps_k{sq}")
                ps_v = ps_big.tile([128, SQ], F32, tag="ps_big", name=f"ps_v{sq}")
                ps_q = ps_big.tile([128, SQ], F32, tag="ps_big", name=f"ps_q{sq}")
                for e in range(NE):
                    for g, ps in ((1, ps_k), (2, ps_v), (0, ps_q)):
                        w_ap = w_sb[:, e * 3 * D + g * D : e * 3 * D + (g + 1) * D]
                        for half in range(2):
                            nc.tensor.matmul(
                                ps[:, half * QB : (half + 1) * QB],
                                lhsT=w_ap,
                                rhs=xt[(sq, e)][:, half * QB : (half + 1) * QB],
                                start=(e == 0),
                                stop=(e == NE - 1),
                            )
                    if sq == 1 and e < 8:
                        transpose_unit(0, e)  # quarter-0 V transposes ride here
                dve_evac(k_all[:, sq * SQ : (sq + 1) * SQ], ps_k, 1)
                vt_tmp[sq] = vtt_pool.tile([128, SQ], BF16, tag="vtt", name=f"vtt{sq}")
                dve_evac(vt_tmp[sq][:], ps_v, 2)
                dve_evac(qT[:, sq * SQ : (sq + 1) * SQ], ps_q, 0)
                if sq == 0:
                    exchange("k0", k_all[:, 0:SQ])

            exchange("v0", v_sb[:, 0 : 8 * D])
            exchange("k1", k_all[:, SQ:HALF])
            # v1's staging DMA is emitted after the stage-0 transposes below

            # peer landings, all on the (otherwise idle) gpsimd queue
            def land_peers():
                nc.gpsimd.dma_start(k_all[:, HALF : HALF + SQ], cc_out["k0"][peer_val])
                nc.gpsimd.dma_start(v_sb[:, 16 * D : 24 * D], cc_out["v0"][peer_val])
                nc.gpsimd.dma_start(k_all[:, HALF + SQ : S], cc_out["k1"][peer_val])
                nc.gpsimd.dma_start(v_sb[:, 24 * D : 32 * D], cc_out["v1"][peer_val])

            # ---- attention: software-pipelined blocks ----
            def scores_exp(qb, kp, on_dve):
                q_ap = qT[:, qb * QB : (qb + 1) * QB]
                ps_s = ps_big.tile([128, 2 * QB], F32, tag="ps_big")
                for half in range(2):
                    k = 2 * kp + half
                    nc.tensor.matmul(
                        ps_s[:, half * QB : (half + 1) * QB],
                        lhsT=k_all[:, k * 128 : (k + 1) * 128],
                        rhs=q_ap,
                        start=True,
                        stop=True,
                    )
                ex = exp_pool.tile([128, 2 * QB], BF16, tag="exp")
                if on_dve:
                    nc.vector.tensor_scalar(
                        ex[:].bitcast(mybir.dt.int16),
                        ps_s[:],
                        SCH_A,
                        SCH_B,
                        mybir.AluOpType.mult,
                        mybir.AluOpType.add,
                    )
                else:
                    nc.scalar.activation(ex[:], ps_s[:], AF.Exp, scale=SCALE)
                return ex

            def subtree(exs):
                level = list(exs)
                while len(level) > 1:
                    nxt = []
                    for i in range(0, len(level), 2):
                        if i + 1 < len(level):
                            comb = comb_pool.tile([128, 2 * QB], BF16, tag="comb")
                            nc.vector.tensor_add(comb[:], level[i][:], level[i + 1][:])
                            nxt.append(comb)
                        else:
                            nxt.append(level[i])
                    level = nxt
                return level[0]

            NKP = 8  # k-pairs per block
            LEAF = 4
            blocks = [(qb, 0) for qb in range(NQB)] + [(qb, 8) for qb in range(NQB)]

            def emit_stage(cur, prev, extra_pe=None):
                """Interleave cur block's scores+exp with prev block's PV.
                extra_pe: optional per-step PE callables (stage-0 transposes).
                """
                if prev is not None:
                    prev["ps_o"] = ps_acc.tile([128, QB], F32, tag="ps_acc", name="ps_o")
                    prev["ps_sum"] = ps_small.tile(
                        [1, QB], F32, tag="ps_small", name="ps_sum"
                    )
                for i in range(NKP):
                    if cur is not None:
                        qb, kp0 = cur["qb"], cur["kp0"]
                        kp = kp0 + i
                        cur["exs"].append(scores_exp(qb, kp, on_dve=(i in (2, 6))))
                        if (i + 1) % LEAF == 0:
                            cur["roots"].append(
                                subtree(cur["exs"][i + 1 - LEAF : i + 1])
                            )
                    if extra_pe is not None and i < len(extra_pe):
                        extra_pe[i]()
                    if prev is not None:
                        kp = prev["kp0"] + i
                        for half in range(2):
                            k = 2 * kp + half
                            nc.tensor.matmul(
                                prev["ps_o"][:],
                                lhsT=v_sb[:, k * D : (k + 1) * D],
                                rhs=prev["exs"][i][:, half * QB : (half + 1) * QB],
                                start=(i == 0 and half == 0),
                                stop=(i == NKP - 1 and half == 1),
                            )
                if prev is None:
                    return
                for ri, root in enumerate(prev["roots"]):
                    for half in range(2):
                        nc.tensor.matmul(
                            prev["ps_sum"][:],
                            lhsT=ones_col[:],
                            rhs=root[:, half * QB : (half + 1) * QB],
                            start=(ri == 0 and half == 0),
                            stop=(ri == len(prev["roots"]) - 1 and half == 1),
                        )
                qb = prev["qb"]
                o_sl = o_stage[:, qb * QB : (qb + 1) * QB]
                s_sl = sums_sb[:, qb * QB : (qb + 1) * QB]
                if prev["kp0"] == 0:  # pass X: stage into SBUF
                    nc.vector.tensor_copy(o_sl, prev["ps_o"][:])
                    nc.vector.tensor_copy(s_sl, prev["ps_sum"][:])
                else:  # pass Y: combine + store
                    o_out = osb_pool.tile([128, QB], F32, tag="osb")
                    nc.vector.tensor_add(o_out[:], o_sl, prev["ps_o"][:])
                    nc.vector.tensor_add(s_sl, s_sl, prev["ps_sum"][:])
                    nc.sync.dma_start(out_d[:, qb * QB : (qb + 1) * QB], o_out[:])
                    nc.sync.dma_start(sums_d[:, qb * QB : (qb + 1) * QB], s_sl)

            prev = None
            for bi, (qb, kp0) in enumerate(blocks):
                cur = {"qb": qb, "kp0": kp0, "exs": [], "roots": []}
                extra = None
                if bi == 0:
                    # quarter-1 V transposes ride the first (PV-less) stage
                    extra = [
                        (lambda j=j: transpose_unit(1, j)) for j in range(8)
                    ]
                emit_stage(cur, prev, extra_pe=extra)
                if bi == 0:
                    exchange("v1", v_sb[:, 8 * D : 16 * D])
                    land_peers()
                prev = cur
            emit_stage(None, prev)

    nc.compile()
    return nc


def _prep_inputs(x, W, b):
    """Host-side sharding prep: cast bf16, transpose to xT, slice halves."""
    b_f = np.asarray(b, dtype=np.float32)
    bias_cols = np.ascontiguousarray(b_f.reshape(3, D).T)  # [128, 3]
    w_bf = np.ascontiguousarray(
        np.asarray(W)
        .astype(ml_dtypes.bfloat16)
        .reshape(E // 128, 128, 3 * D)
        .transpose(1, 0, 2)
        .reshape(128, (E // 128) * 3 * D)
    )
    in_maps = []
    for bb in range(B):
        xt_full = np.ascontiguousarray(
            np.asarray(x[bb]).astype(ml_dtypes.bfloat16).T
        )  # [E, S]
        for h in range(2):
            xc = np.ascontiguousarray(xt_full[:, h * HALF : (h + 1) * HALF])
            peer = np.array([[1 - h]], dtype=np.uint32)
            in_maps.append(
                {"xt": xc, "w": w_bf, "bias_cols": bias_cols, "peer": peer}
            )
    return in_maps


def _run(in_maps, trace=False, trace_kwargs=None):
    if "nc" not in _CACHE:
        _CACHE["nc"] = _build()
    return run_bass_kernel_spmd(
        _CACHE["nc"],
        in_maps,
        list(range(N_CORES)),
        trace=trace,
        **(trace_kwargs or {}),
    )


def kernel(x, W, b):
    in_maps = _prep_inputs(x, W, b)
    res = None
    for attempt in range(3):
        try:
            res = _run(in_maps)
            break
        except Exception:
            if attempt == 2:
                raise
    out = np.empty((B, S, D), dtype=np.float32)
    for c in range(N_CORES):
        bb, h = c // 2, c % 2
        o_t = res.results[c]["out_t"]  # [D, HALF]
        sums = res.results[c]["sums"]  # [1, HALF]
        out[bb, h * HALF : (h + 1) * HALF, :] = (o_t / sums).T
    return out
